# revision 1
# baseline (speedup 1.0000x reference)
"""Self-contained Trainium2 Bass kernel for a 2-layer GCN + FC + log_softmax.

Distribution: nodes sharded across 8 NeuronCores (12500 rows each); edges
partitioned by destination node so each core's scatter-add is local; the
per-layer "gather tables" g = D^-1/2 * H * W are exchanged with an on-chip
AllGather; small weights replicated.

Device algorithm per core:
  Stage A : g1 slice = (dinv*x) @ W1 (rows of this core), bf16 -> AllGather
  Stage B : per 125-node dst tile: PSUM += onehot(seg).T @ g1[src] (banked
            dma_gather of bf16 rows, one-hot M built on VectorE, segment-sum
            as TensorE matmul), + rank-1 sqrt(deg) x b1; then
            h1 = relu(dinv*acc) on ScalarE, g2 = (dinv*h1) @ W2 -> AllGather
  Stage C : same aggregation over g2, then logits = h2 @ Wfc + bfc and a
            fused log_softmax; f32 output slice.
"""
import math

import numpy as np
import ml_dtypes

import concourse.bass as bass
import concourse.mybir as mybir
import concourse.tile as tile
from concourse import bacc, bass_utils

BF16 = ml_dtypes.bfloat16

# Problem contract (hardcoded; must match setup_inputs()).
N_NODES = 100000
N_EDGES = 1600000
D = 128
DOUT = 40

N_CORES = 8
TILE_N = 125            # dst nodes per PSUM tile
BANK = 32768            # gather table bank rows (int16 index limit)
TB = 4                  # dst tiles per gather batch
CHUNK = 128             # edges per matmul chunk
FP32 = mybir.dt.float32
BF16_T = mybir.dt.bfloat16
I16 = mybir.dt.int16
FP16 = mybir.dt.float16


# ---------------------------------------------------------------------------
# Host preprocessing
# ---------------------------------------------------------------------------

def _preprocess(x, edge_index):
    n = x.shape[0]
    slice_n = n // N_CORES
    n_tiles = slice_n // TILE_N
    n_banks = (n + BANK - 1) // BANK

    ei = np.asarray(edge_index, np.int64)
    loops = np.arange(n, dtype=np.int64)
    src = np.concatenate([ei[0], loops])
    dst = np.concatenate([ei[1], loops])
    deg = np.bincount(dst, minlength=n).astype(np.float32)
    dinv = 1.0 / np.sqrt(deg)
    sqdeg = np.sqrt(deg)

    tile_id = dst // TILE_N                       # global tile (core*n_tiles+t)
    batch_g = (tile_id % n_tiles) // TB           # batch within core
    core_id = tile_id // n_tiles
    bank_id = src // BANK
    order = np.lexsort((tile_id, bank_id, batch_g, core_id))
    src_s = src[order]

    batches = [list(range(b, min(b + TB, n_tiles))) for b in range(0, n_tiles, TB)]
    n_batches = len(batches)
    SENT = TB * TILE_N                            # seg sentinel (no tile matches)

    # counts per (core, tile, bank) and per (core, batch, bank)
    key_ctb = (tile_id * n_banks + bank_id)
    cnt_tb = np.bincount(key_ctb, minlength=N_CORES * n_tiles * n_banks).reshape(
        N_CORES, n_tiles, n_banks
    )
    cnt_bb = np.zeros((N_CORES, n_batches, n_banks), np.int64)
    for bi, tiles in enumerate(batches):
        cnt_bb[:, bi, :] = cnt_tb[:, tiles, :].sum(axis=1)
    nbk_bb = ((cnt_bb + CHUNK - 1) // CHUNK).max(axis=0)      # [n_batches, n_banks]

    # group start offsets in the sorted edge array, keyed by (core,batch,bank,tile)
    # sorted order is (core, batch, bank, tile) by the lexsort above
    starts = {}
    pos = 0
    for c in range(N_CORES):
        for bi, tiles in enumerate(batches):
            for k in range(n_banks):
                for t in tiles:
                    m = int(cnt_tb[c, t, k])
                    starts[(c, bi, k, t)] = (pos, m)
                    pos += m
    assert pos == len(src_s)

    # static chunk structure: per batch, per bank, chunk cols; per-tile chunk
    # membership = union over cores
    batch_meta = []
    col = 0
    for bi, tiles in enumerate(batches):
        bm = {"banks": [], "tiles": {t: [] for t in tiles}}
        for k in range(n_banks):
            nbk = int(nbk_bb[bi, k])
            c0 = col
            # per-core tile spans within this (batch, bank) stream
            present = {t: set() for t in tiles}
            for c in range(N_CORES):
                off = 0
                for t in tiles:
                    m = starts[(c, bi, k, t)][1]
                    if m > 0:
                        j0, j1 = off // CHUNK, (off + m - 1) // CHUNK
                        for j in range(j0, j1 + 1):
                            present[t].add(c0 + j)
                    off += m
            for t in tiles:
                for cc in sorted(present[t]):
                    bm["tiles"][t].append((k, cc))
            bm["banks"].append((k, c0, nbk))
            col += nbk
        # order each tile's chunk list by (bank, col) -- already appended that way
        batch_meta.append(bm)
    total_chunks = col

    # Per-core edge arrays in global chunk-column order.
    per_core = []
    for c in range(N_CORES):
        idx_mat = np.zeros((total_chunks, CHUNK), np.int16)
        seg_mat = np.full((total_chunks, CHUNK), SENT, np.int16)
        for bi, tiles in enumerate(batches):
            bm = batch_meta[bi]
            node0 = tiles[0] * TILE_N            # batch-local dst base (per-core)
            for k, c0, nbk in bm["banks"]:
                parts_i, parts_s = [], []
                for t in tiles:
                    s0, m = starts[(c, bi, k, t)]
                    parts_i.append(src_s[s0 : s0 + m] - k * BANK)
                    parts_s.append(dst[order[s0 : s0 + m]] - c * slice_n - node0)
                ei = np.concatenate(parts_i).astype(np.int16)
                es = np.concatenate(parts_s).astype(np.int16)
                m = len(ei)
                pad = nbk * CHUNK - m
                ei = np.concatenate([ei, np.zeros(pad, np.int16)])
                es = np.concatenate([es, np.full(pad, SENT, np.int16)])
                idx_mat[c0 : c0 + nbk] = ei.reshape(nbk, CHUNK)
                seg_mat[c0 : c0 + nbk] = es.reshape(nbk, CHUNK)
        per_core.append((idx_mat, seg_mat))

    meta = {
        "n": n, "slice_n": slice_n, "n_tiles": n_tiles, "n_banks": n_banks,
        "batches": batches, "batch_meta": batch_meta, "total_chunks": total_chunks,
    }
    return per_core, meta, dinv, sqdeg


def _pack_idx(idx_mat, meta):
    """Wrap chunk-major indices into the dma_gather [16, n/16] layout per
    (batch, bank) gather, concatenated along the free dim, replicated to 128
    partitions. Returns ([128, S_total] int16, per-batch [start, len] list)."""
    spans = []
    blocks = []
    s = 0
    for bm in meta["batch_meta"]:
        s0 = s
        for k, c0, nbk in bm["banks"]:
            if nbk == 0:
                continue
            flat = idx_mat[c0 : c0 + nbk].reshape(-1)          # [nbk*128]
            wrapped = flat.reshape(-1, 16).T                   # [16, nbk*8]
            blocks.append(wrapped)
            s += wrapped.shape[1]
        spans.append((s0, s - s0))
    packed = np.concatenate(blocks, axis=1) if blocks else np.zeros((16, 0), np.int16)
    packed = np.tile(packed, (8, 1)).copy()                    # [128, S]
    return packed, spans


def _pack_dinv(v, slice0, meta, dtype):
    """[128, n_tiles]: partition p, col t = v[slice0 + t*TILE_N + p] (p<125)."""
    n_tiles = meta["n_tiles"]
    out = np.zeros((128, n_tiles), dtype)
    sl = v[slice0 : slice0 + n_tiles * TILE_N].reshape(n_tiles, TILE_N)
    out[:TILE_N, :] = sl.T
    return out


# ---------------------------------------------------------------------------
# Device kernel builder
# ---------------------------------------------------------------------------

def _build(meta, s_total, has_bias):
    n = meta["n"]
    slice_n = meta["slice_n"]
    n_tiles = meta["n_tiles"]
    n_banks = meta["n_banks"]
    total_chunks = meta["total_chunks"]
    n_a_tiles = math.ceil(slice_n / 128)

    nc = bacc.Bacc("TRN2", target_bir_lowering=False, debug=False,
                   num_devices=N_CORES, num_swdge_queues=4,
                   dynamic_dma_scratch_size=32768)

    # inputs
    xsT = nc.dram_tensor("xsT", [D, slice_n], BF16_T, kind="ExternalInput")
    w1 = nc.dram_tensor("w1", [D, D], BF16_T, kind="ExternalInput")
    w2 = nc.dram_tensor("w2", [D, D], BF16_T, kind="ExternalInput")
    wfc = nc.dram_tensor("wfc", [D, DOUT], BF16_T, kind="ExternalInput")
    brows = nc.dram_tensor("brows", [4, D], BF16_T, kind="ExternalInput")
    # brows rows: 0=b1, 1=b2, 2=bfc (padded), 3=ones
    iota_in = nc.dram_tensor("iota", [128, TB * TILE_N], FP16, kind="ExternalInput")
    dinvp = nc.dram_tensor("dinvp", [128, n_tiles], FP32, kind="ExternalInput")
    sqdegp = (nc.dram_tensor("sqdegp", [1, slice_n], BF16_T, kind="ExternalInput")
              if has_bias else None)
    idx1 = nc.dram_tensor("idx1", [128, s_total], I16, kind="ExternalInput")
    seg1 = nc.dram_tensor("seg1", [128, total_chunks], FP16, kind="ExternalInput")

    out = nc.dram_tensor("out", [slice_n, DOUT], FP32, kind="ExternalOutput")

    # internal dram
    g1_bounce = nc.dram_tensor("g1_bounce", [slice_n, D], BF16_T)
    g2_bounce = nc.dram_tensor("g2_bounce", [slice_n, D], BF16_T)
    g1_table = nc.dram_tensor("g1_table", [n, D], BF16_T, addr_space="Shared")
    g2_table = nc.dram_tensor("g2_table", [n, D], BF16_T, addr_space="Shared")

    with tile.TileContext(nc) as tc:
        with (
            tc.tile_pool(name="const", bufs=1) as constp,
            tc.tile_pool(name="aio", bufs=4) as aio,
            tc.tile_pool(name="msg", bufs=20) as msgp,
            tc.tile_pool(name="idxp", bufs=8) as idxp,
            tc.tile_pool(name="segp", bufs=8) as segp,
            tc.tile_pool(name="mp", bufs=6) as mp,
            tc.tile_pool(name="fl", bufs=4) as flp,
            tc.tile_pool(name="acc", bufs=3, space="PSUM") as accp,
            tc.tile_pool(name="tps", bufs=2, space="PSUM") as tpsp,
            tc.tile_pool(name="gps", bufs=2, space="PSUM") as gpsp,
        ):
            # constants
            w1_t = constp.tile([D, D], BF16_T, tag="w1")
            nc.sync.dma_start(out=w1_t[:], in_=w1[:, :])
            w2_t = constp.tile([D, D], BF16_T, tag="w2")
            nc.sync.dma_start(out=w2_t[:], in_=w2[:, :])
            wfc_t = constp.tile([D, DOUT], BF16_T, tag="wfc")
            nc.sync.dma_start(out=wfc_t[:], in_=wfc[:, :])
            brow_ts = []
            for r in range(4):
                bt = constp.tile([1, D], BF16_T, tag=f"brow{r}")
                nc.sync.dma_start(out=bt[:], in_=brows[r : r + 1, :])
                brow_ts.append(bt)
            iota_t = constp.tile([128, TB * TILE_N], FP16, tag="iota")
            nc.sync.dma_start(out=iota_t[:], in_=iota_in[:, :])
            dinv_t = constp.tile([128, n_tiles], FP32, tag="dinvp")
            nc.sync.dma_start(out=dinv_t[:], in_=dinvp[:, :])
            if has_bias:
                sqdeg_t = constp.tile([1, slice_n], BF16_T, tag="sqdegp")
                nc.sync.dma_start(out=sqdeg_t[:], in_=sqdegp[:, :])
            ident_t = constp.tile([128, 128], BF16_T, tag="ident")
            from concourse.masks import make_identity
            make_identity(nc, ident_t[:])

            # ---------------- Stage A ----------------
            for t in range(n_a_tiles):
                c0 = t * 128
                w = min(128, slice_n - c0)
                xt = aio.tile([D, 128], BF16_T, tag="xt")
                nc.sync.dma_start(out=xt[:, :w], in_=xsT[:, c0 : c0 + w])
                ps = accp.tile([128, D], FP32, tag="acc")
                nc.tensor.matmul(out=ps[:w, :], lhsT=xt[:, :w], rhs=w1_t[:],
                                 start=True, stop=True)
                gsb = aio.tile([128, D], BF16_T, tag="gsb")
                nc.vector.tensor_copy(out=gsb[:w, :], in_=ps[:w, :])
                nc.sync.dma_start(out=g1_bounce[c0 : c0 + w, :], in_=gsb[:w, :])

            nc.gpsimd.collective_compute(
                "AllGather", mybir.AluOpType.bypass,
                ins=[g1_bounce[:, :]], outs=[g1_table[:, :]],
                replica_groups=[list(range(N_CORES))],
            )

            # ---------------- aggregation layers ----------------
            qrr = [0]
            _regs = {}

            def nidx_reg(v):
                if v not in _regs:
                    _regs[v] = nc.gpsimd.to_reg(v)
                return _regs[v]

            def issue_batch(idx_dram, seg_dram, table, spans, bi, tiles):
                bm = meta["batch_meta"][bi]
                s0, slen = spans[bi]
                it = idxp.tile([128, slen], I16, tag="idx")
                nc.sync.dma_start(out=it[:], in_=idx_dram[:, s0 : s0 + slen])
                allcols = [c for t in tiles for _, c in bm["tiles"][t]]
                cmin, cmax = min(allcols), max(allcols)
                nch_b = cmax - cmin + 1
                st = segp.tile([128, nch_b], FP16, tag="seg")
                nc.sync.dma_start(out=st[:], in_=seg_dram[:, cmin : cmin + nch_b])

                # bank gathers, split to <= MAXC chunks per msg tile
                MAXC = 24
                chunk_map = {}      # global col -> (msg tile, local col)
                off16 = 0
                for k, c0, nbk in bm["banks"]:
                    if nbk == 0:
                        continue
                    rows = min(BANK, n - k * BANK)
                    for g0 in range(0, nbk, MAXC):
                        gl = min(MAXC, nbk - g0)
                        mt = msgp.tile([128, gl, D], BF16_T, tag="msg")
                        nc.gpsimd.dma_gather(
                            out_ap=mt[:],
                            in_ap=table[k * BANK : k * BANK + rows, :],
                            idxs_ap=it[:, off16 : off16 + gl * 8],
                            num_idxs=gl * CHUNK,
                            num_idxs_reg=nidx_reg(gl * CHUNK),
                            elem_size=D,
                            single_packet=False,
                            queue_num=qrr[0] % 4,
                        )
                        qrr[0] += 1
                        for j in range(gl):
                            chunk_map[c0 + g0 + j] = (mt, j)
                        off16 += gl * 8
                return chunk_map, st, cmin

            def agg_layer(idx_dram, seg_dram, table, spans, brow_idx, flush,
                          lookahead=3):
                batches = meta["batches"]
                pending = []
                for bi, tiles in enumerate(batches):
                    pending.append(
                        (bi, tiles,
                         *issue_batch(idx_dram, seg_dram, table, spans, bi, tiles))
                    )
                    if len(pending) > lookahead or bi == len(batches) - 1:
                        todo = pending if bi == len(batches) - 1 else [pending.pop(0)]
                        for job in todo:
                            process_batch(job, brow_idx, flush)
                        if bi == len(batches) - 1:
                            pending = []

            def process_batch(job, brow_idx, flush):
                bi, tiles, chunk_map, st, cmin = job
                bm = meta["batch_meta"][bi]
                if True:
                    for tl, t in enumerate(tiles):
                        acc = accp.tile([128, D], FP32, tag="acc")
                        if has_bias:
                            # rank-1 bias: sqdeg x b
                            nc.tensor.matmul(
                                out=acc[:TILE_N, :],
                                lhsT=sqdeg_t[:, t * TILE_N : (t + 1) * TILE_N],
                                rhs=brow_ts[brow_idx][:, :],
                                start=True, stop=False,
                            )
                        chunk_list = bm["tiles"][t]
                        # group into runs of consecutive cols (<= KM per run)
                        KM = 8
                        runs = []
                        for k, col in chunk_list:
                            if runs and col == runs[-1][0] + len(runs[-1][1]) \
                                    and len(runs[-1][1]) < KM:
                                runs[-1][1].append(col)
                            else:
                                runs.append([col, [col]])
                        nch_t = len(chunk_list)
                        ci = 0
                        for col0, cols_r in runs:
                            r = len(cols_r)
                            mtile = mp.tile([128, r, TILE_N], BF16_T, tag="m")
                            nc.vector.tensor_tensor(
                                out=mtile[:],
                                in0=st[:, col0 - cmin : col0 - cmin + r]
                                    .to_broadcast([128, r, TILE_N]),
                                in1=iota_t[:, tl * TILE_N : (tl + 1) * TILE_N]
                                    .rearrange("p (o x) -> p o x", o=1)
                                    .to_broadcast([128, r, TILE_N]),
                                op=mybir.AluOpType.is_equal,
                            )
                            for i in range(r):
                                mt, j = chunk_map[col0 + i]
                                ci += 1
                                nc.tensor.matmul(
                                    out=acc[:TILE_N, :],
                                    lhsT=mtile[:, i, :],
                                    rhs=mt[:, j, :],
                                    start=(not has_bias and ci == 1),
                                    stop=(ci == nch_t),
                                )
                        flush(t, acc)

            # Stage B flush: h1 = relu(dinv*acc); g2 = (dinv*h1) @ W2
            def flush_b(t, acc):
                dv = dinv_t[:TILE_N, t : t + 1]
                h1 = flp.tile([128, D], BF16_T, tag="h1")
                nc.scalar.activation(out=h1[:TILE_N, :], in_=acc[:TILE_N, :],
                                     func=mybir.ActivationFunctionType.Relu,
                                     scale=dv)
                s = flp.tile([128, D], BF16_T, tag="s")
                nc.scalar.activation(out=s[:TILE_N, :], in_=h1[:TILE_N, :],
                                     func=mybir.ActivationFunctionType.Copy,
                                     scale=dv)
                stp = tpsp.tile([128, TILE_N], BF16_T, tag="stp")
                nc.tensor.transpose(out=stp[:], in_=s[:TILE_N, :],
                                    identity=ident_t[:TILE_N, :TILE_N])
                stb = flp.tile([128, TILE_N], BF16_T, tag="stb")
                nc.vector.tensor_copy(out=stb[:], in_=stp[:])
                g2p = gpsp.tile([128, D], FP32, tag="g2p")
                nc.tensor.matmul(out=g2p[:TILE_N, :], lhsT=stb[:],
                                 rhs=w2_t[:], start=True, stop=True)
                g2sb = flp.tile([128, D], BF16_T, tag="g2sb")
                nc.vector.tensor_copy(out=g2sb[:TILE_N, :], in_=g2p[:TILE_N, :])
                nc.scalar.dma_start(
                    out=g2_bounce[t * TILE_N : (t + 1) * TILE_N, :],
                    in_=g2sb[:TILE_N, :],
                )

            # Stage C flush: h2 = relu(dinv*acc); logits; log_softmax
            def flush_c(t, acc):
                dv = dinv_t[:TILE_N, t : t + 1]
                h2 = flp.tile([128, D], BF16_T, tag="h1")
                nc.scalar.activation(out=h2[:TILE_N, :], in_=acc[:TILE_N, :],
                                     func=mybir.ActivationFunctionType.Relu,
                                     scale=dv)
                htp = tpsp.tile([128, TILE_N], BF16_T, tag="stp")
                nc.tensor.transpose(out=htp[:], in_=h2[:TILE_N, :],
                                    identity=ident_t[:TILE_N, :TILE_N])
                htb = flp.tile([128, TILE_N], BF16_T, tag="stb")
                nc.vector.tensor_copy(out=htb[:], in_=htp[:])
                lg = gpsp.tile([128, DOUT], FP32, tag="g2p")
                nc.tensor.matmul(out=lg[:TILE_N, :], lhsT=htb[:],
                                 rhs=wfc_t[:], start=True, stop=not has_bias)
                if has_bias:
                    nc.tensor.matmul(out=lg[:TILE_N, :],
                                     lhsT=brow_ts[3][:, :TILE_N],
                                     rhs=brow_ts[2][:, :DOUT],
                                     start=False, stop=True)
                mx = flp.tile([128, 1], FP32, tag="mx")
                nc.vector.tensor_reduce(out=mx[:TILE_N, :], in_=lg[:TILE_N, :],
                                        axis=mybir.AxisListType.X,
                                        op=mybir.AluOpType.max)
                negm = flp.tile([128, 1], FP32, tag="negm")
                nc.vector.tensor_scalar_mul(out=negm[:TILE_N, :],
                                            in0=mx[:TILE_N, :], scalar1=-1.0)
                esc = flp.tile([128, DOUT], BF16_T, tag="esc")
                ssum = flp.tile([128, 1], FP32, tag="ssum")
                nc.scalar.activation(out=esc[:TILE_N, :], in_=lg[:TILE_N, :],
                                     func=mybir.ActivationFunctionType.Exp,
                                     bias=negm[:TILE_N, :],
                                     accum_out=ssum[:TILE_N, :])
                lns = flp.tile([128, 1], FP32, tag="lns")
                nc.scalar.activation(out=lns[:TILE_N, :], in_=ssum[:TILE_N, :],
                                     func=mybir.ActivationFunctionType.Ln)
                nmls = flp.tile([128, 1], FP32, tag="nmls")
                nc.vector.tensor_tensor(out=nmls[:TILE_N, :],
                                        in0=negm[:TILE_N, :],
                                        in1=lns[:TILE_N, :],
                                        op=mybir.AluOpType.subtract)
                ot = flp.tile([128, DOUT], FP32, tag="ot")
                nc.vector.tensor_tensor(out=ot[:TILE_N, :], in0=lg[:TILE_N, :],
                                        in1=nmls[:TILE_N, :]
                                            .to_broadcast([TILE_N, DOUT]),
                                        op=mybir.AluOpType.add)
                nc.scalar.dma_start(
                    out=out[t * TILE_N : (t + 1) * TILE_N, :],
                    in_=ot[:TILE_N, :],
                )

            import os
            dbg = os.environ.get("GCN_DEBUG", "")

            def flush_b_min(t, acc):
                dv = dinv_t[:TILE_N, t : t + 1]
                h1 = flp.tile([128, D], BF16_T, tag="h1")
                nc.scalar.activation(out=h1[:TILE_N, :], in_=acc[:TILE_N, :],
                                     func=mybir.ActivationFunctionType.Relu,
                                     scale=dv)
                nc.sync.dma_start(
                    out=g2_bounce[t * TILE_N : (t + 1) * TILE_N, :],
                    in_=h1[:TILE_N, :],
                )

            if dbg != "A":
                agg_layer(idx1, seg1, g1_table, meta["spans1"], 0,
                          flush_b_min if dbg == "Bmin" else flush_b)
                if dbg not in ("B", "Bmin"):
                    nc.gpsimd.collective_compute(
                        "AllGather", mybir.AluOpType.bypass,
                        ins=[g2_bounce[:, :]], outs=[g2_table[:, :]],
                        replica_groups=[list(range(N_CORES))],
                    )
                    if dbg == "Cmin":
                        agg_layer(idx1, seg1, g2_table, meta["spans2"], 1,
                                  flush_b_min)
                    elif dbg != "B2":
                        agg_layer(idx1, seg1, g2_table, meta["spans2"], 1, flush_c)

    nc.compile()
    return nc


# ---------------------------------------------------------------------------
# Public entry point
# ---------------------------------------------------------------------------

_CACHE = {}


def kernel(x, edge_index, W1, b1, W2, b2, Wfc, bfc):
    x = np.asarray(x, np.float32)
    per_core, meta, dinv, sqdeg = _preprocess(x, edge_index)
    n, slice_n = meta["n"], meta["slice_n"]

    # same static structure for both layers -> same spans
    idx_packs, seg_packs = [], []
    spans = None
    for c in range(N_CORES):
        idx_mat, seg_mat = per_core[c]
        packed, spans = _pack_idx(idx_mat, meta)
        idx_packs.append(packed)
        seg_packs.append(np.ascontiguousarray(seg_mat.T).astype(np.float16))
    meta["spans1"] = spans
    meta["spans2"] = spans
    s_total = idx_packs[0].shape[1]

    has_bias = bool(
        np.any(np.asarray(b1)) or np.any(np.asarray(b2)) or np.any(np.asarray(bfc))
    )
    key = (n, meta["total_chunks"], s_total, has_bias)
    if key not in _CACHE:
        _CACHE[key] = _build(meta, s_total, has_bias)
    nc = _CACHE[key]

    xs = (dinv[:, None] * x).T           # [D, n]
    xsT_b = np.ascontiguousarray(xs).astype(BF16)
    iota = np.tile(np.arange(TB * TILE_N, dtype=np.float16), (128, 1))
    brows = np.zeros((4, D), np.float32)
    brows[0, :] = np.asarray(b1, np.float32)
    brows[1, :] = np.asarray(b2, np.float32)
    brows[2, : DOUT] = np.asarray(bfc, np.float32)
    brows[3, :] = 1.0
    brows = brows.astype(BF16)

    in_maps = []
    for c in range(N_CORES):
        s0 = c * slice_n
        im = {
            "xsT": np.ascontiguousarray(xsT_b[:, s0 : s0 + slice_n]),
            "w1": np.asarray(W1, np.float32).astype(BF16),
            "w2": np.asarray(W2, np.float32).astype(BF16),
            "wfc": np.asarray(Wfc, np.float32).astype(BF16),
            "brows": brows,
            "iota": iota,
            "dinvp": _pack_dinv(dinv, s0, meta, np.float32),
            "idx1": idx_packs[c],
            "seg1": seg_packs[c],
        }
        if has_bias:
            im["sqdegp"] = sqdeg[s0 : s0 + slice_n][None, :].astype(BF16)
        in_maps.append(im)

    global _last_in_maps
    _last_in_maps = in_maps
    last_exc = None
    for _attempt in range(3):
        try:
            res = bass_utils.run_bass_kernel_spmd(
                nc, in_maps, core_ids=list(range(N_CORES))
            )
            return np.concatenate(
                [res.results[c]["out"] for c in range(N_CORES)], axis=0
            )
        except Exception as e:  # transient device/tunnel errors: retry
            last_exc = e
    raise last_exc


_last_in_maps = None



# revision 5
# speedup vs baseline: 1.9888x; 1.9888x over previous
"""Self-contained Trainium2 Bass kernel for a 2-layer GCN + FC + log_softmax.

Distribution: nodes sharded across 8 NeuronCores (12500 rows each); edges
partitioned by destination node so each core's scatter-add is local; the
per-layer "gather tables" g = D^-1/2 * H * W are exchanged with an on-chip
AllGather; small weights replicated.

Device algorithm per core:
  Stage A : g1 slice = (dinv*x) @ W1 (rows of this core), fp16 -> AllGather
  Agg     : per 125-node dst tile: PSUM += onehot(seg).T @ g1[src]
            (dma_gather of fp16 rows from 4 HBM banks on 4 SWDGE queues,
            one-hot built on VectorE from preloaded seg values, segment-sum
            as TensorE matmul).  Self-loops are NOT gathered: they are a
            diag(dinv) matmul against the core-local g rows.
  Flush B : h1 = relu(dinv*acc); g2 = (dinv*h1) @ W2 -> AllGather
  Flush C : h2 = relu(dinv*acc); logits = h2 @ Wfc; fused log_softmax.
"""
import hashlib
import math

import numpy as np
import ml_dtypes

import concourse.bass as bass
import concourse.mybir as mybir
import concourse.tile as tile
from concourse import bacc, bass_utils

FP16_NP = ml_dtypes.float16 if hasattr(ml_dtypes, "float16") else np.float16

# Problem contract (hardcoded; must match setup_inputs()).
N_NODES = 100000
N_EDGES = 1600000
D = 128
DOUT = 40

N_CORES = 8
TILE_N = 125            # dst nodes per PSUM tile
TB = 10                 # dst tiles per batch
BANK = 25000            # gather table bank rows (int16 index limit 32767)
CHUNK = 128             # edges per matmul chunk
MAXC = 24               # chunks per dma_gather call
KM = 8                  # max chunks per one-hot build run
AHEAD = 2               # gather issue lookahead (batches)
PAR_OFF = 500.0         # seg offset for odd tiles (collision guard)
SENT = 1250.0           # seg sentinel (matches no iota value)

FP32 = mybir.dt.float32
FP16 = mybir.dt.float16
I16 = mybir.dt.int16

SLICE_N = N_NODES // N_CORES            # 12500
N_TILES = SLICE_N // TILE_N             # 100
N_BATCH = N_TILES // TB                 # 10
N_BANKS = (N_NODES + BANK - 1) // BANK  # 4


# ---------------------------------------------------------------------------
# Host preprocessing
# ---------------------------------------------------------------------------

def _preprocess(edge_index):
    """Sort edges by (core, batch, bank, tile); build the static chunk
    structure shared by both conv layers plus per-core idx/seg arrays."""
    ei = np.asarray(edge_index, np.int64)
    src, dst = ei[0], ei[1]
    deg = (np.bincount(dst, minlength=N_NODES) + 1.0).astype(np.float32)
    dinv = 1.0 / np.sqrt(deg)
    sqdeg = np.sqrt(deg)

    tile_id = dst // TILE_N                  # global tile 0..799
    core_id = tile_id // N_TILES
    t_loc = tile_id % N_TILES                # tile within core 0..99
    batch_g = t_loc // TB
    bank_id = src // BANK
    order = np.lexsort((tile_id, bank_id, batch_g, core_id))
    src_s = src[order]
    dst_s = dst[order]
    tloc_s = t_loc[order]

    # per (core, batch, bank, tile-in-batch) counts
    tl_in_b = tloc_s % TB
    key = (((core_id[order] * N_BATCH + batch_g[order]) * N_BANKS
            + bank_id[order]) * TB + tl_in_b)
    cnt4 = np.bincount(key, minlength=N_CORES * N_BATCH * N_BANKS * TB)
    cnt4 = cnt4.reshape(N_CORES, N_BATCH, N_BANKS, TB)
    cnt3 = cnt4.sum(axis=3)                                # [C, B, K]
    nbk = (-(-cnt3 // CHUNK)).max(axis=0)                  # [B, K] chunks

    # chunk columns + per-tile chunk ranges (union over cores)
    col0 = np.zeros((N_BATCH, N_BANKS), np.int64)
    c = 0
    for b in range(N_BATCH):
        for k in range(N_BANKS):
            col0[b, k] = c
            c += int(nbk[b, k])
    total_chunks = c

    ends = np.cumsum(cnt4, axis=3)                         # [C,B,K,TB]
    starts = ends - cnt4
    lo = np.where(cnt4 > 0, starts // CHUNK, np.iinfo(np.int64).max)
    hi = np.where(cnt4 > 0, -(-ends // CHUNK), 0)
    lo = lo.min(axis=0)                                    # [B,K,TB]
    hi = hi.max(axis=0)

    # runs per (batch, tile): contiguous chunk ranges split to <= KM
    tile_runs = {}
    for b in range(N_BATCH):
        for tl in range(TB):
            runs = []
            for k in range(N_BANKS):
                l, h = int(lo[b, k, tl]), int(hi[b, k, tl])
                if h <= l:
                    continue
                base = int(col0[b, k])
                x = l
                while x < h:
                    r = min(KM, h - x)
                    runs.append((base + x, r))
                    x += r
            tile_runs[(b, tl)] = runs

    # gather call pieces
    pieces = []                      # (b, k, row0, rows, gl, col_start, off16)
    batch_pieces = [[] for _ in range(N_BATCH)]
    s16 = 0
    for b in range(N_BATCH):
        for k in range(N_BANKS):
            nb = int(nbk[b, k])
            rows = min(BANK, N_NODES - k * BANK)
            g0 = 0
            while g0 < nb:
                gl = min(MAXC, nb - g0)
                batch_pieces[b].append(len(pieces))
                pieces.append((b, k, k * BANK, rows, gl,
                               int(col0[b, k]) + g0, s16))
                s16 += gl * 8
                g0 += gl
    s_total = s16

    # per-core idx/seg arrays in chunk-column order
    per_core = []
    seg_all = ((dst_s % TILE_N) + PAR_OFF * (tloc_s % 2)).astype(np.float16)
    idx_all = (src_s - bank_id[order] * BANK).astype(np.int16)
    # group start offset per (core, batch, bank) in sorted edge order
    grp_sizes = cnt3.reshape(-1)
    grp_off = np.concatenate([[0], np.cumsum(grp_sizes)])
    for cidx in range(N_CORES):
        idx_mat = np.zeros((total_chunks, CHUNK), np.int16)
        seg_mat = np.full((total_chunks, CHUNK), SENT, np.float16)
        for b in range(N_BATCH):
            for k in range(N_BANKS):
                g = (cidx * N_BATCH + b) * N_BANKS + k
                o0, m = int(grp_off[g]), int(grp_sizes[g])
                nb = int(nbk[b, k])
                ii = np.zeros(nb * CHUNK, np.int16)
                ss = np.full(nb * CHUNK, SENT, np.float16)
                ii[:m] = idx_all[o0 : o0 + m]
                ss[:m] = seg_all[o0 : o0 + m]
                c0 = int(col0[b, k])
                idx_mat[c0 : c0 + nb] = ii.reshape(nb, CHUNK)
                seg_mat[c0 : c0 + nb] = ss.reshape(nb, CHUNK)
        per_core.append((idx_mat, seg_mat))

    meta = {
        "total_chunks": total_chunks, "s_total": s_total,
        "pieces": pieces, "batch_pieces": batch_pieces,
        "tile_runs": tile_runs,
    }
    return per_core, meta, dinv, sqdeg


def _pack_idx(idx_mat, meta):
    """Wrap chunk-major indices into the dma_gather [16, n/16] layout per
    (batch, bank) block, concatenated, replicated to 128 partitions."""
    blocks = []
    for b, k, row0, rows, gl, cs, off16 in meta["pieces"]:
        flat = idx_mat[cs : cs + gl].reshape(-1)           # [gl*128]
        blocks.append(flat.reshape(-1, 16).T)              # [16, gl*8]
    packed = np.concatenate(blocks, axis=1)
    assert packed.shape[1] == meta["s_total"]
    return np.tile(packed, (8, 1)).copy()                  # [128, S]


def _pack_dinv(v, slice0, dtype):
    """[128, n_tiles]: partition p, col t = v[slice0 + t*TILE_N + p]."""
    out = np.zeros((128, N_TILES), dtype)
    sl = v[slice0 : slice0 + N_TILES * TILE_N].reshape(N_TILES, TILE_N)
    out[:TILE_N, :] = sl.T
    return out


# ---------------------------------------------------------------------------
# Device kernel builder
# ---------------------------------------------------------------------------

def _build(meta, has_bias):
    total_chunks = meta["total_chunks"]
    s_total = meta["s_total"]
    pieces = meta["pieces"]
    batch_pieces = meta["batch_pieces"]
    tile_runs = meta["tile_runs"]
    n_a_tiles = math.ceil(SLICE_N / 128)

    nc = bacc.Bacc("TRN2", target_bir_lowering=False, debug=False,
                   num_devices=N_CORES, num_swdge_queues=4,
                   dynamic_dma_scratch_size=32768)

    # inputs
    xsT = nc.dram_tensor("xsT", [D, SLICE_N], FP16, kind="ExternalInput")
    w1 = nc.dram_tensor("w1", [D, D], FP16, kind="ExternalInput")
    w2 = nc.dram_tensor("w2", [D, D], FP16, kind="ExternalInput")
    wfc = nc.dram_tensor("wfc", [D, DOUT], FP16, kind="ExternalInput")
    iota_in = nc.dram_tensor("iota", [128, 2 * KM * TILE_N], FP16,
                             kind="ExternalInput")
    dinvp = nc.dram_tensor("dinvp", [128, N_TILES], FP32, kind="ExternalInput")
    idx1 = nc.dram_tensor("idx1", [128, s_total], I16, kind="ExternalInput")
    seg1 = nc.dram_tensor("seg1", [128, total_chunks], FP16,
                          kind="ExternalInput")
    brows = (nc.dram_tensor("brows", [4, D], FP16, kind="ExternalInput")
             if has_bias else None)
    sqdegp = (nc.dram_tensor("sqdegp", [1, SLICE_N], FP16,
                             kind="ExternalInput") if has_bias else None)

    out = nc.dram_tensor("out", [SLICE_N, DOUT], FP32, kind="ExternalOutput")

    # internal dram
    g1_bounce = nc.dram_tensor("g1_bounce", [SLICE_N, D], FP16)
    g2_bounce = nc.dram_tensor("g2_bounce", [SLICE_N, D], FP16)
    g1_table = nc.dram_tensor("g1_table", [N_NODES, D], FP16,
                              addr_space="Shared")
    g2_table = nc.dram_tensor("g2_table", [N_NODES, D], FP16,
                              addr_space="Shared")

    with tile.TileContext(nc) as tc:
        with (
            tc.tile_pool(name="const", bufs=1) as constp,
            tc.tile_pool(name="aio", bufs=4) as aio,
            tc.tile_pool(name="msg", bufs=16) as msgp,
            tc.tile_pool(name="mp", bufs=8) as mp,
            tc.tile_pool(name="gs", bufs=4) as gsp,
            tc.tile_pool(name="fl", bufs=4) as flp,
            tc.tile_pool(name="acc", bufs=4, space="PSUM") as accp,
            tc.tile_pool(name="tps", bufs=2, space="PSUM") as tpsp,
            tc.tile_pool(name="gps", bufs=2, space="PSUM") as gpsp,
        ):
            # constants
            w1_t = constp.tile([D, D], FP16, tag="w1")
            nc.sync.dma_start(out=w1_t[:], in_=w1[:, :])
            w2_t = constp.tile([D, D], FP16, tag="w2")
            nc.sync.dma_start(out=w2_t[:], in_=w2[:, :])
            wfc_t = constp.tile([D, DOUT], FP16, tag="wfc")
            nc.sync.dma_start(out=wfc_t[:], in_=wfc[:, :])
            iota_t = constp.tile([128, 2 * KM * TILE_N], FP16, tag="iota")
            nc.sync.dma_start(out=iota_t[:], in_=iota_in[:, :])
            dinv_t = constp.tile([128, N_TILES], FP32, tag="dinvp")
            nc.sync.dma_start(out=dinv_t[:], in_=dinvp[:, :])
            idx_t = constp.tile([128, s_total], I16, tag="idx")
            nc.sync.dma_start(out=idx_t[:], in_=idx1[:, :])
            seg_t = constp.tile([128, total_chunks], FP16, tag="seg")
            nc.sync.dma_start(out=seg_t[:], in_=seg1[:, :])
            if has_bias:
                brow_ts = []
                for r in range(4):
                    bt = constp.tile([1, D], FP16, tag=f"brow{r}")
                    nc.sync.dma_start(out=bt[:], in_=brows[r : r + 1, :])
                    brow_ts.append(bt)
                sqdeg_t = constp.tile([1, SLICE_N], FP16, tag="sqdegp")
                nc.sync.dma_start(out=sqdeg_t[:], in_=sqdegp[:, :])
            ident_t = constp.tile([128, 128], FP16, tag="ident")
            from concourse.masks import make_identity
            make_identity(nc, ident_t[:])

            # ---------------- Stage A ----------------
            for t in range(n_a_tiles):
                c0 = t * 128
                w = min(128, SLICE_N - c0)
                xt = aio.tile([D, 128], FP16, tag="xt")
                nc.sync.dma_start(out=xt[:, :w], in_=xsT[:, c0 : c0 + w])
                ps = accp.tile([128, D], FP32, tag="acc")
                nc.tensor.matmul(out=ps[:w, :], lhsT=xt[:, :w], rhs=w1_t[:],
                                 start=True, stop=True)
                gsb = aio.tile([128, D], FP16, tag="gsb")
                nc.vector.tensor_copy(out=gsb[:w, :], in_=ps[:w, :])
                nc.sync.dma_start(out=g1_bounce[c0 : c0 + w, :], in_=gsb[:w, :])

            nc.gpsimd.collective_compute(
                "AllGather", mybir.AluOpType.bypass,
                ins=[g1_bounce[:, :]], outs=[g1_table[:, :]],
                replica_groups=[list(range(N_CORES))],
            )

            # ---------------- aggregation layers ----------------
            qrr = [0]
            _regs = {}

            def nidx_reg(v):
                if v not in _regs:
                    _regs[v] = nc.gpsimd.to_reg(v)
                return _regs[v]

            def issue_batch(table, bi):
                cmap = {}
                for pi in batch_pieces[bi]:
                    b, k, row0, rows, gl, cs, off16 = pieces[pi]
                    mt = msgp.tile([128, gl, D], FP16, tag="msg")
                    nc.gpsimd.dma_gather(
                        out_ap=mt[:],
                        in_ap=table[row0 : row0 + rows, :],
                        idxs_ap=idx_t[:, off16 : off16 + gl * 8],
                        num_idxs=gl * CHUNK,
                        num_idxs_reg=nidx_reg(gl * CHUNK),
                        elem_size=D,
                        single_packet=False,
                        queue_num=qrr[0] % 4,
                    )
                    qrr[0] += 1
                    for j in range(gl):
                        cmap[cs + j] = (mt, j)
                return cmap

            def process_batch(bi, cmap, bounce, brow_idx, flush):
                for tl in range(TB):
                    tg = bi * TB + tl
                    t0 = tg * TILE_N
                    acc = accp.tile([128, D], FP32, tag="acc")
                    # self-loop: identity @ g_local (flush's *dinv[d] covers
                    # the dinv[d]^2 * h[d] = dinv[d] * g[d] self message)
                    gself = gsp.tile([128, D], FP16, tag="gself")
                    nc.sync.dma_start(out=gself[:TILE_N, :],
                                      in_=bounce[t0 : t0 + TILE_N, :])
                    nc.tensor.matmul(out=acc[:TILE_N, :],
                                     lhsT=ident_t[:TILE_N, :TILE_N],
                                     rhs=gself[:TILE_N, :],
                                     start=True, stop=False)
                    if has_bias:
                        nc.tensor.matmul(
                            out=acc[:TILE_N, :],
                            lhsT=sqdeg_t[:, t0 : t0 + TILE_N],
                            rhs=brow_ts[brow_idx][:, :],
                            start=False, stop=False,
                        )
                    runs = tile_runs[(bi, tl)]
                    nch = sum(r for _, r in runs)
                    par = tl % 2
                    io0 = par * KM * TILE_N
                    ci = 0
                    for col0, r in runs:
                        mtile = mp.tile([128, r, TILE_N], FP16, tag="m")
                        nc.vector.tensor_tensor(
                            out=mtile[:],
                            in0=seg_t[:, col0 : col0 + r]
                                .rearrange("p (r o) -> p r o", o=1)
                                .to_broadcast([128, r, TILE_N]),
                            in1=iota_t[:, io0 : io0 + r * TILE_N]
                                .rearrange("p (r x) -> p r x", r=r),
                            op=mybir.AluOpType.is_equal,
                        )
                        for i in range(r):
                            mt, j = cmap[col0 + i]
                            ci += 1
                            nc.tensor.matmul(
                                out=acc[:TILE_N, :],
                                lhsT=mtile[:, i, :],
                                rhs=mt[:, j, :],
                                start=False,
                                stop=(ci == nch),
                            )
                    flush(tg, acc)

            def agg_layer(table, bounce, brow_idx, flush):
                issued = {}
                for bi in range(min(AHEAD + 1, N_BATCH)):
                    issued[bi] = issue_batch(table, bi)
                for bi in range(N_BATCH):
                    process_batch(bi, issued.pop(bi), bounce, brow_idx, flush)
                    nxt = bi + AHEAD + 1
                    if nxt < N_BATCH:
                        issued[nxt] = issue_batch(table, nxt)

            # Flush B: h1 = relu(dinv*acc); g2 = (dinv*h1) @ W2
            def flush_b(tg, acc):
                dv = dinv_t[:TILE_N, tg : tg + 1]
                h1 = flp.tile([128, D], FP16, tag="h1")
                nc.scalar.activation(out=h1[:TILE_N, :], in_=acc[:TILE_N, :],
                                     func=mybir.ActivationFunctionType.Relu,
                                     scale=dv)
                s = flp.tile([128, D], FP16, tag="s")
                nc.scalar.activation(out=s[:TILE_N, :], in_=h1[:TILE_N, :],
                                     func=mybir.ActivationFunctionType.Copy,
                                     scale=dv)
                stp = tpsp.tile([128, TILE_N], FP16, tag="stp")
                nc.tensor.transpose(out=stp[:], in_=s[:TILE_N, :],
                                    identity=ident_t[:TILE_N, :TILE_N])
                stb = flp.tile([128, TILE_N], FP16, tag="stb")
                nc.vector.tensor_copy(out=stb[:], in_=stp[:])
                g2p = gpsp.tile([128, D], FP32, tag="g2p")
                nc.tensor.matmul(out=g2p[:TILE_N, :], lhsT=stb[:],
                                 rhs=w2_t[:], start=True, stop=True)
                g2sb = flp.tile([128, D], FP16, tag="g2sb")
                nc.vector.tensor_copy(out=g2sb[:TILE_N, :], in_=g2p[:TILE_N, :])
                nc.scalar.dma_start(
                    out=g2_bounce[tg * TILE_N : (tg + 1) * TILE_N, :],
                    in_=g2sb[:TILE_N, :],
                )

            # Flush C: h2 = relu(dinv*acc); logits; log_softmax
            def flush_c(tg, acc):
                dv = dinv_t[:TILE_N, tg : tg + 1]
                h2 = flp.tile([128, D], FP16, tag="h1")
                nc.scalar.activation(out=h2[:TILE_N, :], in_=acc[:TILE_N, :],
                                     func=mybir.ActivationFunctionType.Relu,
                                     scale=dv)
                htp = tpsp.tile([128, TILE_N], FP16, tag="stp")
                nc.tensor.transpose(out=htp[:], in_=h2[:TILE_N, :],
                                    identity=ident_t[:TILE_N, :TILE_N])
                htb = flp.tile([128, TILE_N], FP16, tag="stb")
                nc.vector.tensor_copy(out=htb[:], in_=htp[:])
                lg = gpsp.tile([128, DOUT], FP32, tag="g2p")
                nc.tensor.matmul(out=lg[:TILE_N, :], lhsT=htb[:],
                                 rhs=wfc_t[:], start=True, stop=not has_bias)
                if has_bias:
                    nc.tensor.matmul(out=lg[:TILE_N, :],
                                     lhsT=brow_ts[3][:, :TILE_N],
                                     rhs=brow_ts[2][:, :DOUT],
                                     start=False, stop=True)
                mx = flp.tile([128, 1], FP32, tag="mx")
                nc.vector.tensor_reduce(out=mx[:TILE_N, :], in_=lg[:TILE_N, :],
                                        axis=mybir.AxisListType.X,
                                        op=mybir.AluOpType.max)
                negm = flp.tile([128, 1], FP32, tag="negm")
                nc.vector.tensor_scalar_mul(out=negm[:TILE_N, :],
                                            in0=mx[:TILE_N, :], scalar1=-1.0)
                esc = flp.tile([128, DOUT], FP16, tag="esc")
                ssum = flp.tile([128, 1], FP32, tag="ssum")
                nc.scalar.activation(out=esc[:TILE_N, :], in_=lg[:TILE_N, :],
                                     func=mybir.ActivationFunctionType.Exp,
                                     bias=negm[:TILE_N, :],
                                     accum_out=ssum[:TILE_N, :])
                lns = flp.tile([128, 1], FP32, tag="lns")
                nc.scalar.activation(out=lns[:TILE_N, :], in_=ssum[:TILE_N, :],
                                     func=mybir.ActivationFunctionType.Ln)
                nmls = flp.tile([128, 1], FP32, tag="nmls")
                nc.vector.tensor_tensor(out=nmls[:TILE_N, :],
                                        in0=negm[:TILE_N, :],
                                        in1=lns[:TILE_N, :],
                                        op=mybir.AluOpType.subtract)
                ot = flp.tile([128, DOUT], FP32, tag="ot")
                nc.vector.tensor_tensor(out=ot[:TILE_N, :], in0=lg[:TILE_N, :],
                                        in1=nmls[:TILE_N, :]
                                            .to_broadcast([TILE_N, DOUT]),
                                        op=mybir.AluOpType.add)
                nc.scalar.dma_start(
                    out=out[tg * TILE_N : (tg + 1) * TILE_N, :],
                    in_=ot[:TILE_N, :],
                )

            agg_layer(g1_table, g1_bounce, 0, flush_b)
            nc.gpsimd.collective_compute(
                "AllGather", mybir.AluOpType.bypass,
                ins=[g2_bounce[:, :]], outs=[g2_table[:, :]],
                replica_groups=[list(range(N_CORES))],
            )
            agg_layer(g2_table, g2_bounce, 1, flush_c)

    nc.compile()
    return nc


# ---------------------------------------------------------------------------
# Public entry point
# ---------------------------------------------------------------------------

_CACHE = {}


def kernel(x, edge_index, W1, b1, W2, b2, Wfc, bfc):
    x = np.asarray(x, np.float32)
    per_core, meta, dinv, sqdeg = _preprocess(edge_index)

    has_bias = bool(
        np.any(np.asarray(b1)) or np.any(np.asarray(b2)) or np.any(np.asarray(bfc))
    )
    mkey = hashlib.sha1(
        repr((meta["total_chunks"], meta["s_total"], meta["pieces"],
              sorted(meta["tile_runs"].items()), has_bias)).encode()
    ).hexdigest()
    if mkey not in _CACHE:
        _CACHE[mkey] = _build(meta, has_bias)
    nc = _CACHE[mkey]

    xs = (dinv[:, None] * x).T           # [D, n]
    xsT_h = np.ascontiguousarray(xs).astype(np.float16)
    # iota: even tiles use values j%125 at cols [0, KM*125); odd tiles use
    # 500 + j%125 at cols [KM*125, 2*KM*125)
    base = np.tile(np.arange(TILE_N, dtype=np.float32), KM)
    iota = np.concatenate([base, base + PAR_OFF]).astype(np.float16)
    iota = np.tile(iota, (128, 1))
    if has_bias:
        brows_np = np.zeros((4, D), np.float32)
        brows_np[0, :] = np.asarray(b1, np.float32)
        brows_np[1, :] = np.asarray(b2, np.float32)
        brows_np[2, : DOUT] = np.asarray(bfc, np.float32)
        brows_np[3, :] = 1.0
        brows_np = brows_np.astype(np.float16)

    in_maps = []
    for c in range(N_CORES):
        s0 = c * SLICE_N
        idx_mat, seg_mat = per_core[c]
        im = {
            "xsT": np.ascontiguousarray(xsT_h[:, s0 : s0 + SLICE_N]),
            "w1": np.asarray(W1, np.float32).astype(np.float16),
            "w2": np.asarray(W2, np.float32).astype(np.float16),
            "wfc": np.asarray(Wfc, np.float32).astype(np.float16),
            "iota": iota,
            "dinvp": _pack_dinv(dinv, s0, np.float32),
            "idx1": _pack_idx(idx_mat, meta),
            "seg1": np.ascontiguousarray(seg_mat.T),
        }
        if has_bias:
            im["brows"] = brows_np
            im["sqdegp"] = sqdeg[s0 : s0 + SLICE_N][None, :].astype(np.float16)
        in_maps.append(im)

    global _last_in_maps
    _last_in_maps = in_maps
    last_exc = None
    for _attempt in range(3):
        try:
            res = bass_utils.run_bass_kernel_spmd(
                nc, in_maps, core_ids=list(range(N_CORES))
            )
            return np.concatenate(
                [res.results[c]["out"] for c in range(N_CORES)], axis=0
            )
        except Exception as e:  # transient device/tunnel errors: retry
            last_exc = e
    raise last_exc


_last_in_maps = None


# revision 9
# speedup vs baseline: 2.1756x; 1.0939x over previous
"""Self-contained Trainium2 Bass kernel for a 2-layer GCN + FC + log_softmax.

Distribution: nodes sharded across 8 NeuronCores (12500 rows each); edges
partitioned by destination node so each core's scatter-add is local; the
per-layer "gather tables" g = D^-1/2 * H * W are exchanged with an on-chip
AllGather; small weights replicated.

Device algorithm per core:
  Stage A : g1 slice = (dinv*x) @ W1 (rows of this core), fp16 -> AllGather
  Agg     : per 125-node dst tile: PSUM += onehot(seg).T @ g1[src]
            (dma_gather of fp16 rows from 4 HBM banks on 4 SWDGE queues,
            one-hot built on VectorE from preloaded seg values, segment-sum
            as TensorE matmul).  Self-loops are NOT gathered: they are a
            diag(dinv) matmul against the core-local g rows.
  Flush B : h1 = relu(dinv*acc); g2 = (dinv*h1) @ W2 -> AllGather
  Flush C : h2 = relu(dinv*acc); logits = h2 @ Wfc; fused log_softmax.
"""
import hashlib
import math

import numpy as np
import ml_dtypes

import concourse.bass as bass
import concourse.mybir as mybir
import concourse.tile as tile
from concourse import bacc, bass_utils

FP16_NP = ml_dtypes.float16 if hasattr(ml_dtypes, "float16") else np.float16

# Problem contract (hardcoded; must match setup_inputs()).
N_NODES = 100000
N_EDGES = 1600000
D = 128
DOUT = 40

N_CORES = 8
TILE_N = 125            # dst nodes per PSUM tile
TB = 10                 # dst tiles per batch
BANK = 25000            # gather table bank rows (int16 index limit 32767)
CHUNK = 128             # edges per matmul chunk
MAXC = 24               # chunks per dma_gather call
KM = 8                  # max chunks per one-hot build run
AHEAD = 2               # gather issue lookahead (batches)
PAR_OFF = 500.0         # seg offset for odd tiles (collision guard)
SENT = 1250.0           # seg sentinel (matches no iota value)

FP32 = mybir.dt.float32
FP16 = mybir.dt.float16
I16 = mybir.dt.int16

SLICE_N = N_NODES // N_CORES            # 12500
N_TILES = SLICE_N // TILE_N             # 100
N_BATCH = N_TILES // TB                 # 10
N_BANKS = (N_NODES + BANK - 1) // BANK  # 4


# ---------------------------------------------------------------------------
# Host preprocessing
# ---------------------------------------------------------------------------

def _preprocess(edge_index):
    """Sort edges by (core, batch, bank, tile); build the static chunk
    structure shared by both conv layers plus per-core idx/seg arrays."""
    ei = np.asarray(edge_index, np.int64)
    src, dst = ei[0], ei[1]
    deg = (np.bincount(dst, minlength=N_NODES) + 1.0).astype(np.float32)
    dinv = 1.0 / np.sqrt(deg)
    sqdeg = np.sqrt(deg)

    tile_id = dst // TILE_N                  # global tile 0..799
    core_id = tile_id // N_TILES
    t_loc = tile_id % N_TILES                # tile within core 0..99
    batch_g = t_loc // TB
    bank_id = src // BANK
    order = np.lexsort((tile_id, bank_id, batch_g, core_id))
    src_s = src[order]
    dst_s = dst[order]
    tloc_s = t_loc[order]

    # per (core, batch, bank, tile-in-batch) counts
    tl_in_b = tloc_s % TB
    key = (((core_id[order] * N_BATCH + batch_g[order]) * N_BANKS
            + bank_id[order]) * TB + tl_in_b)
    cnt4 = np.bincount(key, minlength=N_CORES * N_BATCH * N_BANKS * TB)
    cnt4 = cnt4.reshape(N_CORES, N_BATCH, N_BANKS, TB)
    cnt3 = cnt4.sum(axis=3)                                # [C, B, K]
    nbk = (-(-cnt3 // CHUNK)).max(axis=0)                  # [B, K] chunks

    # chunk columns + per-tile chunk ranges (union over cores)
    col0 = np.zeros((N_BATCH, N_BANKS), np.int64)
    c = 0
    for b in range(N_BATCH):
        for k in range(N_BANKS):
            col0[b, k] = c
            c += int(nbk[b, k])
    total_chunks = c

    ends = np.cumsum(cnt4, axis=3)                         # [C,B,K,TB]
    starts = ends - cnt4
    lo = np.where(cnt4 > 0, starts // CHUNK, np.iinfo(np.int64).max)
    hi = np.where(cnt4 > 0, -(-ends // CHUNK), 0)
    lo = lo.min(axis=0)                                    # [B,K,TB]
    hi = hi.max(axis=0)

    # runs per (batch, tile): contiguous chunk ranges split to <= KM
    tile_runs = {}
    for b in range(N_BATCH):
        for tl in range(TB):
            runs = []
            for k in range(N_BANKS):
                l, h = int(lo[b, k, tl]), int(hi[b, k, tl])
                if h <= l:
                    continue
                base = int(col0[b, k])
                x = l
                while x < h:
                    r = min(KM, h - x)
                    runs.append((base + x, r))
                    x += r
            tile_runs[(b, tl)] = runs

    # gather call pieces: split each (batch, bank) into two equal halves so
    # the round-robin SWDGE queues see balanced generation work
    pieces = []                      # (b, k, row0, rows, gl, col_start, off16)
    batch_pieces = [[] for _ in range(N_BATCH)]
    s16 = 0
    for b in range(N_BATCH):
        for k in range(N_BANKS):
            nb = int(nbk[b, k])
            rows = min(BANK, N_NODES - k * BANK)
            nsplit = max(1, -(-nb // MAXC))
            g0 = 0
            for si in range(nsplit):
                gl = (nb - g0) // (nsplit - si)
                if gl == 0:
                    continue
                batch_pieces[b].append(len(pieces))
                pieces.append((b, k, k * BANK, rows, gl,
                               int(col0[b, k]) + g0, s16))
                s16 += gl * 8
                g0 += gl
    s_total = s16

    # per-core idx/seg arrays in chunk-column order
    per_core = []
    seg_all = ((dst_s % TILE_N) + PAR_OFF * (tloc_s % 2)).astype(np.float16)
    idx_all = (src_s - bank_id[order] * BANK).astype(np.int16)
    # group start offset per (core, batch, bank) in sorted edge order
    grp_sizes = cnt3.reshape(-1)
    grp_off = np.concatenate([[0], np.cumsum(grp_sizes)])
    for cidx in range(N_CORES):
        idx_mat = np.zeros((total_chunks, CHUNK), np.int16)
        seg_mat = np.full((total_chunks, CHUNK), SENT, np.float16)
        for b in range(N_BATCH):
            for k in range(N_BANKS):
                g = (cidx * N_BATCH + b) * N_BANKS + k
                o0, m = int(grp_off[g]), int(grp_sizes[g])
                nb = int(nbk[b, k])
                ii = np.zeros(nb * CHUNK, np.int16)
                ss = np.full(nb * CHUNK, SENT, np.float16)
                ii[:m] = idx_all[o0 : o0 + m]
                ss[:m] = seg_all[o0 : o0 + m]
                c0 = int(col0[b, k])
                idx_mat[c0 : c0 + nb] = ii.reshape(nb, CHUNK)
                seg_mat[c0 : c0 + nb] = ss.reshape(nb, CHUNK)
        per_core.append((idx_mat, seg_mat))

    meta = {
        "total_chunks": total_chunks, "s_total": s_total,
        "pieces": pieces, "batch_pieces": batch_pieces,
        "tile_runs": tile_runs,
    }
    return per_core, meta, dinv, sqdeg


def _pack_idx(idx_mat, meta):
    """Wrap chunk-major indices into the dma_gather [16, n/16] layout per
    (batch, bank) block, concatenated, replicated to 128 partitions."""
    blocks = []
    for b, k, row0, rows, gl, cs, off16 in meta["pieces"]:
        flat = idx_mat[cs : cs + gl].reshape(-1)           # [gl*128]
        blocks.append(flat.reshape(-1, 16).T)              # [16, gl*8]
    packed = np.concatenate(blocks, axis=1)
    assert packed.shape[1] == meta["s_total"]
    return np.tile(packed, (8, 1)).copy()                  # [128, S]


def _pack_dinv(v, slice0, dtype):
    """[128, n_tiles]: partition p, col t = v[slice0 + t*TILE_N + p]."""
    out = np.zeros((128, N_TILES), dtype)
    sl = v[slice0 : slice0 + N_TILES * TILE_N].reshape(N_TILES, TILE_N)
    out[:TILE_N, :] = sl.T
    return out


# ---------------------------------------------------------------------------
# Device kernel builder
# ---------------------------------------------------------------------------

def _build(meta, has_bias):
    total_chunks = meta["total_chunks"]
    s_total = meta["s_total"]
    pieces = meta["pieces"]
    batch_pieces = meta["batch_pieces"]
    tile_runs = meta["tile_runs"]
    n_a_tiles = math.ceil(SLICE_N / 128)

    nc = bacc.Bacc("TRN2", target_bir_lowering=False, debug=False,
                   num_devices=N_CORES, num_swdge_queues=4,
                   dynamic_dma_scratch_size=32768)

    # inputs
    xsT = nc.dram_tensor("xsT", [D, SLICE_N], FP16, kind="ExternalInput")
    w1 = nc.dram_tensor("w1", [D, D], FP16, kind="ExternalInput")
    w2 = nc.dram_tensor("w2", [D, D], FP16, kind="ExternalInput")
    wfc = nc.dram_tensor("wfc", [D, DOUT], FP16, kind="ExternalInput")
    iota_in = nc.dram_tensor("iota", [128, 2 * KM * TILE_N], FP16,
                             kind="ExternalInput")
    dinvp = nc.dram_tensor("dinvp", [128, N_TILES], FP32, kind="ExternalInput")
    dinv2p = nc.dram_tensor("dinv2p", [128, N_TILES], FP32,
                            kind="ExternalInput")
    idx1 = nc.dram_tensor("idx1", [128, s_total], I16, kind="ExternalInput")
    seg1 = nc.dram_tensor("seg1", [128, total_chunks], FP16,
                          kind="ExternalInput")
    brows = (nc.dram_tensor("brows", [4, D], FP16, kind="ExternalInput")
             if has_bias else None)
    sqdegp = (nc.dram_tensor("sqdegp", [1, SLICE_N], FP16,
                             kind="ExternalInput") if has_bias else None)

    out = nc.dram_tensor("out", [SLICE_N, DOUT], FP32, kind="ExternalOutput")

    # internal dram
    g1_bounce = nc.dram_tensor("g1_bounce", [SLICE_N, D], FP16)
    g2_bounce = nc.dram_tensor("g2_bounce", [SLICE_N, D], FP16)
    g1_table = nc.dram_tensor("g1_table", [N_NODES, D], FP16,
                              addr_space="Shared")
    g2_table = nc.dram_tensor("g2_table", [N_NODES, D], FP16,
                              addr_space="Shared")

    with tile.TileContext(nc) as tc:
        with (
            tc.tile_pool(name="const", bufs=1) as constp,
            tc.tile_pool(name="aio", bufs=4) as aio,
            tc.tile_pool(name="msg", bufs=24) as msgp,
            tc.tile_pool(name="mp", bufs=6) as mp,
            tc.tile_pool(name="gs", bufs=4) as gsp,
            tc.tile_pool(name="fl", bufs=4) as flp,
            tc.tile_pool(name="lg", bufs=12) as lgp,
            tc.tile_pool(name="nm", bufs=12) as nmp,
            tc.tile_pool(name="acc", bufs=4, space="PSUM") as accp,
            tc.tile_pool(name="tps", bufs=2, space="PSUM") as tpsp,
            tc.tile_pool(name="gps", bufs=2, space="PSUM") as gpsp,
        ):
            # constants
            w1_t = constp.tile([D, D], FP16, tag="w1")
            nc.sync.dma_start(out=w1_t[:], in_=w1[:, :])
            w2_t = constp.tile([D, D], FP16, tag="w2")
            nc.sync.dma_start(out=w2_t[:], in_=w2[:, :])
            wfc_t = constp.tile([D, DOUT], FP16, tag="wfc")
            nc.sync.dma_start(out=wfc_t[:], in_=wfc[:, :])
            iota_t = constp.tile([128, 2 * KM * TILE_N], FP16, tag="iota")
            nc.sync.dma_start(out=iota_t[:], in_=iota_in[:, :])
            dinv_t = constp.tile([128, N_TILES], FP32, tag="dinvp")
            nc.sync.dma_start(out=dinv_t[:], in_=dinvp[:, :])
            dinv2_t = constp.tile([128, N_TILES], FP32, tag="dinv2p")
            nc.sync.dma_start(out=dinv2_t[:], in_=dinv2p[:, :])
            idx_t = constp.tile([128, s_total], I16, tag="idx")
            nc.sync.dma_start(out=idx_t[:], in_=idx1[:, :])
            seg_t = constp.tile([128, total_chunks], FP16, tag="seg")
            nc.sync.dma_start(out=seg_t[:], in_=seg1[:, :])
            if has_bias:
                brow_ts = []
                for r in range(4):
                    bt = constp.tile([1, D], FP16, tag=f"brow{r}")
                    nc.sync.dma_start(out=bt[:], in_=brows[r : r + 1, :])
                    brow_ts.append(bt)
                sqdeg_t = constp.tile([1, SLICE_N], FP16, tag="sqdegp")
                nc.sync.dma_start(out=sqdeg_t[:], in_=sqdegp[:, :])
            ident_t = constp.tile([128, 128], FP16, tag="ident")
            from concourse.masks import make_identity
            make_identity(nc, ident_t[:])

            # ---------------- Stage A ----------------
            BL = 4
            nblk = math.ceil(SLICE_N / (128 * BL))
            for bb in range(nblk):
                c0 = bb * 128 * BL
                bw = min(128 * BL, SLICE_N - c0)
                nch = math.ceil(bw / 128)
                xt = aio.tile([D, 128 * BL], FP16, tag="xt")
                nc.sync.dma_start(out=xt[:, :bw], in_=xsT[:, c0 : c0 + bw])
                gsb = aio.tile([128, BL, D], FP16, tag="gsb")
                for i in range(nch):
                    w = min(128, bw - i * 128)
                    ps = accp.tile([128, D], FP32, tag="acc")
                    nc.tensor.matmul(out=ps[:w, :],
                                     lhsT=xt[:, i * 128 : i * 128 + w],
                                     rhs=w1_t[:], start=True, stop=True)
                    nc.vector.tensor_copy(out=gsb[:w, i, :], in_=ps[:w, :])
                full = (bw // 128) * 128
                if full:
                    nc.sync.dma_start(
                        out=g1_bounce[c0 : c0 + full, :]
                            .rearrange("(b p) d -> p b d", p=128),
                        in_=gsb[:, : full // 128, :])
                if bw > full:
                    w = bw - full
                    nc.sync.dma_start(
                        out=g1_bounce[c0 + full : c0 + bw, :],
                        in_=gsb[:w, full // 128, :])

            nc.gpsimd.collective_compute(
                "AllGather", mybir.AluOpType.bypass,
                ins=[g1_bounce[:, :]], outs=[g1_table[:, :]],
                replica_groups=[list(range(N_CORES))],
            )

            # ---------------- aggregation layers ----------------
            qrr = [0]
            _regs = {}

            def nidx_reg(v):
                if v not in _regs:
                    _regs[v] = nc.gpsimd.to_reg(v)
                return _regs[v]

            def issue_batch(table, bi):
                cmap = {}
                for pi in batch_pieces[bi]:
                    b, k, row0, rows, gl, cs, off16 = pieces[pi]
                    mt = msgp.tile([128, gl, D], FP16, tag="msg")
                    nc.gpsimd.dma_gather(
                        out_ap=mt[:],
                        in_ap=table[row0 : row0 + rows, :],
                        idxs_ap=idx_t[:, off16 : off16 + gl * 8],
                        num_idxs=gl * CHUNK,
                        num_idxs_reg=nidx_reg(gl * CHUNK),
                        elem_size=D,
                        single_packet=False,
                        queue_num=qrr[0] % 4,
                    )
                    qrr[0] += 1
                    for j in range(gl):
                        cmap[cs + j] = (mt, j)
                return cmap

            def process_batch(bi, cmap, bounce, brow_idx, flush):
                for tl in range(TB):
                    tg = bi * TB + tl
                    t0 = tg * TILE_N
                    acc = accp.tile([128, D], FP32, tag="acc")
                    # self-loop: identity @ g_local (flush's *dinv[d] covers
                    # the dinv[d]^2 * h[d] = dinv[d] * g[d] self message)
                    gself = gsp.tile([128, D], FP16, tag="gself")
                    nc.sync.dma_start(out=gself[:TILE_N, :],
                                      in_=bounce[t0 : t0 + TILE_N, :])
                    nc.tensor.matmul(out=acc[:TILE_N, :],
                                     lhsT=ident_t[:TILE_N, :TILE_N],
                                     rhs=gself[:TILE_N, :],
                                     start=True, stop=False)
                    if has_bias:
                        nc.tensor.matmul(
                            out=acc[:TILE_N, :],
                            lhsT=sqdeg_t[:, t0 : t0 + TILE_N],
                            rhs=brow_ts[brow_idx][:, :],
                            start=False, stop=False,
                        )
                    runs = tile_runs[(bi, tl)]
                    nch = sum(r for _, r in runs)
                    par = tl % 2
                    io0 = par * KM * TILE_N
                    ci = 0
                    for col0, r in runs:
                        mtile = mp.tile([128, r, TILE_N], FP16, tag="m")
                        nc.vector.tensor_tensor(
                            out=mtile[:],
                            in0=seg_t[:, col0 : col0 + r]
                                .rearrange("p (r o) -> p r o", o=1)
                                .to_broadcast([128, r, TILE_N]),
                            in1=iota_t[:, io0 : io0 + r * TILE_N]
                                .rearrange("p (r x) -> p r x", r=r),
                            op=mybir.AluOpType.is_equal,
                        )
                        for i in range(r):
                            mt, j = cmap[col0 + i]
                            ci += 1
                            nc.tensor.matmul(
                                out=acc[:TILE_N, :],
                                lhsT=mtile[:, i, :],
                                rhs=mt[:, j, :],
                                start=False,
                                stop=(ci == nch),
                            )
                    flush(tg, acc)

            def agg_layer(table, bounce, brow_idx, flush, post_batch=None):
                issued = {}
                for bi in range(min(AHEAD + 1, N_BATCH)):
                    issued[bi] = issue_batch(table, bi)
                for bi in range(N_BATCH):
                    process_batch(bi, issued.pop(bi), bounce, brow_idx, flush)
                    if post_batch is not None:
                        post_batch()
                    nxt = bi + AHEAD + 1
                    if nxt < N_BATCH:
                        issued[nxt] = issue_batch(table, nxt)

            # Flush B: s = dinv*relu(dinv*acc) = relu(dinv^2*acc);
            # g2 = s @ W2
            def flush_b(tg, acc):
                dv2 = dinv2_t[:TILE_N, tg : tg + 1]
                s = flp.tile([128, D], FP16, tag="s")
                nc.scalar.activation(out=s[:TILE_N, :], in_=acc[:TILE_N, :],
                                     func=mybir.ActivationFunctionType.Relu,
                                     scale=dv2)
                stp = tpsp.tile([128, TILE_N], FP16, tag="stp")
                nc.tensor.transpose(out=stp[:], in_=s[:TILE_N, :],
                                    identity=ident_t[:TILE_N, :TILE_N])
                stb = flp.tile([128, TILE_N], FP16, tag="stb")
                nc.vector.tensor_copy(out=stb[:], in_=stp[:])
                g2p = gpsp.tile([128, D], FP32, tag="g2p")
                nc.tensor.matmul(out=g2p[:TILE_N, :], lhsT=stb[:],
                                 rhs=w2_t[:], start=True, stop=True)
                g2sb = flp.tile([128, D], FP16, tag="g2sb")
                nc.vector.tensor_copy(out=g2sb[:TILE_N, :], in_=g2p[:TILE_N, :])
                nc.scalar.dma_start(
                    out=g2_bounce[tg * TILE_N : (tg + 1) * TILE_N, :],
                    in_=g2sb[:TILE_N, :],
                )

            # Flush C phase 1: h2 = relu(dinv*acc); logits -> SBUF; max.
            # Exp/Ln are deferred to post_batch_c so the scalar engine runs
            # them in homogeneous groups (2 activation-table loads per batch
            # instead of 2 per tile).
            c_pend = []

            def flush_c(tg, acc):
                dv = dinv_t[:TILE_N, tg : tg + 1]
                h2 = flp.tile([128, D], FP16, tag="h1")
                nc.scalar.activation(out=h2[:TILE_N, :], in_=acc[:TILE_N, :],
                                     func=mybir.ActivationFunctionType.Relu,
                                     scale=dv)
                htp = tpsp.tile([128, TILE_N], FP16, tag="stp")
                nc.tensor.transpose(out=htp[:], in_=h2[:TILE_N, :],
                                    identity=ident_t[:TILE_N, :TILE_N])
                htb = flp.tile([128, TILE_N], FP16, tag="stb")
                nc.vector.tensor_copy(out=htb[:], in_=htp[:])
                lg = gpsp.tile([128, DOUT], FP32, tag="g2p")
                nc.tensor.matmul(out=lg[:TILE_N, :], lhsT=htb[:],
                                 rhs=wfc_t[:], start=True, stop=not has_bias)
                if has_bias:
                    nc.tensor.matmul(out=lg[:TILE_N, :],
                                     lhsT=brow_ts[3][:, :TILE_N],
                                     rhs=brow_ts[2][:, :DOUT],
                                     start=False, stop=True)
                lgs = lgp.tile([128, DOUT], FP32, tag="lgs")
                nc.vector.tensor_copy(out=lgs[:TILE_N, :], in_=lg[:TILE_N, :])
                mx = flp.tile([128, 1], FP32, tag="mx")
                nc.vector.tensor_reduce(out=mx[:TILE_N, :], in_=lg[:TILE_N, :],
                                        axis=mybir.AxisListType.X,
                                        op=mybir.AluOpType.max)
                negm = nmp.tile([128, 1], FP32, tag="negm")
                nc.vector.tensor_scalar_mul(out=negm[:TILE_N, :],
                                            in0=mx[:TILE_N, :], scalar1=-1.0)
                c_pend.append((tg, lgs, negm))

            def post_batch_c():
                sums = []
                for tg, lgs, negm in c_pend:
                    esc = flp.tile([128, DOUT], FP16, tag="esc")
                    ssum = nmp.tile([128, 1], FP32, tag="ssum")
                    nc.scalar.activation(out=esc[:TILE_N, :],
                                         in_=lgs[:TILE_N, :],
                                         func=mybir.ActivationFunctionType.Exp,
                                         bias=negm[:TILE_N, :],
                                         accum_out=ssum[:TILE_N, :])
                    sums.append(ssum)
                for (tg, lgs, negm), ssum in zip(c_pend, sums):
                    lns = flp.tile([128, 1], FP32, tag="lns")
                    nc.scalar.activation(out=lns[:TILE_N, :],
                                         in_=ssum[:TILE_N, :],
                                         func=mybir.ActivationFunctionType.Ln)
                    nmls = flp.tile([128, 1], FP32, tag="nmls")
                    nc.vector.tensor_tensor(out=nmls[:TILE_N, :],
                                            in0=negm[:TILE_N, :],
                                            in1=lns[:TILE_N, :],
                                            op=mybir.AluOpType.subtract)
                    ot = flp.tile([128, DOUT], FP32, tag="ot")
                    nc.vector.tensor_tensor(out=ot[:TILE_N, :],
                                            in0=lgs[:TILE_N, :],
                                            in1=nmls[:TILE_N, :]
                                                .to_broadcast([TILE_N, DOUT]),
                                            op=mybir.AluOpType.add)
                    nc.scalar.dma_start(
                        out=out[tg * TILE_N : (tg + 1) * TILE_N, :],
                        in_=ot[:TILE_N, :],
                    )
                c_pend.clear()

            agg_layer(g1_table, g1_bounce, 0, flush_b)
            nc.gpsimd.collective_compute(
                "AllGather", mybir.AluOpType.bypass,
                ins=[g2_bounce[:, :]], outs=[g2_table[:, :]],
                replica_groups=[list(range(N_CORES))],
            )
            agg_layer(g2_table, g2_bounce, 1, flush_c, post_batch_c)

    nc.compile()
    return nc


# ---------------------------------------------------------------------------
# Public entry point
# ---------------------------------------------------------------------------

_CACHE = {}


def kernel(x, edge_index, W1, b1, W2, b2, Wfc, bfc):
    x = np.asarray(x, np.float32)
    per_core, meta, dinv, sqdeg = _preprocess(edge_index)

    has_bias = bool(
        np.any(np.asarray(b1)) or np.any(np.asarray(b2)) or np.any(np.asarray(bfc))
    )
    mkey = hashlib.sha1(
        repr((meta["total_chunks"], meta["s_total"], meta["pieces"],
              sorted(meta["tile_runs"].items()), has_bias)).encode()
    ).hexdigest()
    if mkey not in _CACHE:
        _CACHE[mkey] = _build(meta, has_bias)
    nc = _CACHE[mkey]

    xs = (dinv[:, None] * x).T           # [D, n]
    xsT_h = np.ascontiguousarray(xs).astype(np.float16)
    # iota: even tiles use values j%125 at cols [0, KM*125); odd tiles use
    # 500 + j%125 at cols [KM*125, 2*KM*125)
    base = np.tile(np.arange(TILE_N, dtype=np.float32), KM)
    iota = np.concatenate([base, base + PAR_OFF]).astype(np.float16)
    iota = np.tile(iota, (128, 1))
    if has_bias:
        brows_np = np.zeros((4, D), np.float32)
        brows_np[0, :] = np.asarray(b1, np.float32)
        brows_np[1, :] = np.asarray(b2, np.float32)
        brows_np[2, : DOUT] = np.asarray(bfc, np.float32)
        brows_np[3, :] = 1.0
        brows_np = brows_np.astype(np.float16)

    in_maps = []
    for c in range(N_CORES):
        s0 = c * SLICE_N
        idx_mat, seg_mat = per_core[c]
        im = {
            "xsT": np.ascontiguousarray(xsT_h[:, s0 : s0 + SLICE_N]),
            "w1": np.asarray(W1, np.float32).astype(np.float16),
            "w2": np.asarray(W2, np.float32).astype(np.float16),
            "wfc": np.asarray(Wfc, np.float32).astype(np.float16),
            "iota": iota,
            "dinvp": _pack_dinv(dinv, s0, np.float32),
            "dinv2p": _pack_dinv(dinv * dinv, s0, np.float32),
            "idx1": _pack_idx(idx_mat, meta),
            "seg1": np.ascontiguousarray(seg_mat.T),
        }
        if has_bias:
            im["brows"] = brows_np
            im["sqdegp"] = sqdeg[s0 : s0 + SLICE_N][None, :].astype(np.float16)
        in_maps.append(im)

    global _last_in_maps
    _last_in_maps = in_maps
    last_exc = None
    for _attempt in range(3):
        try:
            res = bass_utils.run_bass_kernel_spmd(
                nc, in_maps, core_ids=list(range(N_CORES))
            )
            return np.concatenate(
                [res.results[c]["out"] for c in range(N_CORES)], axis=0
            )
        except Exception as e:  # transient device/tunnel errors: retry
            last_exc = e
    raise last_exc


_last_in_maps = None
